# revision 9
# baseline (speedup 1.0000x reference)
"""Causal self-attention (dense transformer block) on 8 Trainium2 NeuronCores.

Sharding (Megatron-style tensor parallel over heads):
  - 16 heads, 8 cores -> 2 heads/core. Each core computes qkv projection for
    its 2 heads (column-sharded W_qkv), causal attention for those heads over
    all 4 batches, and a row-sharded c_proj partial (its 128 y-channels x
    W_proj rows). Host sums the 8 partial outputs (the row-parallel unshard).
  - Everything on device is kept in transposed [channels, rows] layout so no
    on-device transposes of activations are needed; only V is transposed
    (per 128x128 tile on the PE) to feed the PV matmul as [keys, ch].
  - Softmax: scores are O(+-6) so exp() without max-subtraction is exact in
    fp32; row sums come free from the PV matmul via a ones-column appended to
    V ([V|1]); causal masking is a 0/1 multiply on the 4 diagonal k-tiles.
  - All matmuls run in float32r (TF32-like, 11-bit mantissa, 4x the fp32
    rate). Inputs are pre-rounded to f32r on the host; PSUM accumulation is
    full fp32.
"""

import sys

sys.path.insert(0, "/opt/trn_rl_repo")

import numpy as np

N_CORES = 8
B, T, D = 4, 2048, 1024
H, DK = 16, 64
HPC = H // N_CORES            # heads per core = 2
CPC = HPC * DK                # channels per core = 128
ROWS = B * T                  # 8192
RT = 512                      # row-tile (free dim) for projections
N_RT = ROWS // RT             # 16
KTILE = 128                   # key tile
QB = 512                      # query block
N_QB = T // QB                # 4 query blocks per batch
N_KT_B = T // KTILE           # 16 key tiles per batch
SCALE = 1.0 / np.sqrt(DK)


def round_f32r(x):
    """Round fp32 -> fp32r (round-to-nearest-even at 11 fraction bits)."""
    b = np.ascontiguousarray(x, dtype=np.float32).view(np.uint32)
    r = ((b.astype(np.uint64) + 0x7FF + ((b >> 12) & 1)) & ~np.uint64(0xFFF)).astype(
        np.uint32
    )
    return r.view(np.float32)


def build_program():
    import concourse.bass as bass  # noqa: F401
    import concourse.mybir as mybir
    import concourse.tile as tile
    from concourse import bacc
    from concourse.masks import make_identity

    f32 = mybir.dt.float32
    f32r = mybir.dt.float32r
    ACTF = mybir.ActivationFunctionType
    MUL = mybir.AluOpType.mult

    nc = bacc.Bacc(None, target_bir_lowering=False)
    with tile.TileContext(nc) as tc:
        with tc.tile_pool(name="dram", bufs=1, space="DRAM") as dram:
            xT = dram.tile([D, ROWS], f32r, kind="ExternalInput", name="xT", uniquify=False)
            wq = dram.tile([D, CPC], f32r, kind="ExternalInput", name="wq", uniquify=False)
            wk = dram.tile([D, CPC], f32r, kind="ExternalInput", name="wk", uniquify=False)
            wv = dram.tile([D, CPC], f32r, kind="ExternalInput", name="wv", uniquify=False)
            wplo = dram.tile([DK, D], f32r, kind="ExternalInput", name="wplo", uniquify=False)
            wphi = dram.tile([DK, D], f32r, kind="ExternalInput", name="wphi", uniquify=False)
            bqkv = dram.tile([CPC, 3], f32, kind="ExternalInput", name="bqkv", uniquify=False)
            bp = dram.tile([128, D // 128], f32, kind="ExternalInput", name="bp", uniquify=False)
            outT = dram.tile([D, ROWS], f32, kind="ExternalOutput", name="outT", uniquify=False)

            # ---------------- constants / weights in SBUF ----------------
            cst = tc.alloc_tile_pool(name="cst", bufs=1)
            # weight tiles: [128 (contraction chunk), kt * M] layout
            wq_sb = cst.tile([128, D], f32r, name="wq_sb")
            wk_sb = cst.tile([128, D], f32r, name="wk_sb")
            wv_sb = cst.tile([128, D], f32r, name="wv_sb")
            for w_dram, w_sb in ((wq, wq_sb), (wk, wk_sb), (wv, wv_sb)):
                nc.sync.dma_start(
                    out=w_sb[:].rearrange("p (t m) -> p t m", m=CPC),
                    in_=w_dram[:].rearrange("(t p) m -> p t m", p=128),
                )
            wplo_sb = cst.tile([DK, D], f32r, name="wplo_sb")
            wphi_sb = cst.tile([DK, D], f32r, name="wphi_sb")
            nc.sync.dma_start(out=wplo_sb[:], in_=wplo[:])
            nc.sync.dma_start(out=wphi_sb[:], in_=wphi[:])
            bqkv_sb = cst.tile([CPC, 3], f32, name="bqkv_sb")
            nc.sync.dma_start(out=bqkv_sb[:], in_=bqkv[:])
            bp_sb = cst.tile([128, D // 128], f32, name="bp_sb")
            nc.sync.dma_start(out=bp_sb[:], in_=bp[:])

            ones32 = cst.tile([128, 1], f32, name="ones32")
            nc.vector.memset(ones32[:], 1.0)

            ident32 = cst.tile([128, 128], f32, name="ident32")
            make_identity(nc, ident32)
            ident = cst.tile([128, 128], f32r, name="ident")
            nc.vector.tensor_copy(ident[:], ident32[:])

            # 4 diagonal causal masks [128 k, 512 q]: keep where q >= k + off
            msk = cst.tile([128, 4 * QB], f32r, name="msk")
            mscratch = cst.tile([128, QB], f32, name="mscratch")
            for j in range(4):
                nc.gpsimd.memset(mscratch[:], 1.0)
                nc.gpsimd.affine_select(
                    out=mscratch[:],
                    in_=mscratch[:],
                    compare_op=mybir.AluOpType.is_ge,
                    fill=0.0,
                    base=-(j * 128),
                    pattern=[[1, QB]],
                    channel_multiplier=-1,
                )
                nc.vector.tensor_copy(msk[:, j * QB:(j + 1) * QB], mscratch[:])

            # ---------------- long-lived activations ----------------
            qt_sb, _free_qt = tc.tile([CPC, ROWS], f32r, name="qt_sb")
            kt_sb, _free_kt = tc.tile([CPC, ROWS], f32r, name="kt_sb")
            # V tiles: per key-tile g: [128 keys, 130]: h0 V|1 at cols 0:65,
            # h1 V|1 at cols 65:130
            v_sb, _free_v = tc.tile([128, (ROWS // KTILE) * 130], f32r, name="v_sb")

            # ---------------- pools ----------------
            xa = tc.alloc_tile_pool(name="xa", bufs=10)
            vts = tc.alloc_tile_pool(name="vts", bufs=3)
            att = tc.alloc_tile_pool(name="att", bufs=5)
            ynp = tc.alloc_tile_pool(name="ynp", bufs=2)
            bcp = tc.alloc_tile_pool(name="bcp", bufs=2)
            osp = tc.alloc_tile_pool(name="osp", bufs=3)
            rrp = tc.alloc_tile_pool(name="rrp", bufs=2)
            ps_gp = tc.alloc_tile_pool(name="ps_gp", bufs=4, space="PSUM")
            ps_s = tc.alloc_tile_pool(name="ps_s", bufs=2, space="PSUM")
            ps_acc = tc.alloc_tile_pool(name="ps_acc", bufs=2, space="PSUM")

            # ================= phase 1: qkv projections =================
            for rt in range(N_RT):
                rsl = slice(rt * RT, (rt + 1) * RT)
                xts = []
                for kt in range(D // 128):
                    xt = xa.tile([128, RT], f32r, name="xt", tag="xt")
                    nc.sync.dma_start(out=xt[:], in_=xT[kt * 128:(kt + 1) * 128, rsl])
                    xts.append(xt)
                p_q = ps_gp.tile([CPC, RT], f32, name="p_q", tag="ps")
                p_k = ps_gp.tile([CPC, RT], f32, name="p_k", tag="ps")
                p_v = ps_gp.tile([CPC, RT], f32, name="p_v", tag="ps")
                nkt = D // 128
                for kt in range(nkt):
                    ksl = slice(kt * 128, (kt + 1) * 128)
                    st = kt == 0
                    sp = kt == nkt - 1
                    nc.tensor.matmul(p_q[:], wq_sb[:, ksl], xts[kt][:], start=st, stop=sp)
                    nc.tensor.matmul(p_k[:], wk_sb[:, ksl], xts[kt][:], start=st, stop=sp)
                    nc.tensor.matmul(p_v[:], wv_sb[:, ksl], xts[kt][:], start=st, stop=sp)
                # evict Q^T, K^T with bias
                nc.vector.tensor_scalar_add(qt_sb[:, rsl], p_q[:], bqkv_sb[:, 0:1])
                nc.vector.tensor_scalar_add(kt_sb[:, rsl], p_k[:], bqkv_sb[:, 1:2])
                # V^T -> SBUF (with bias), then PE-transpose into V tiles
                vt_t = vts.tile([CPC, RT], f32r, name="vt_t", tag="vt")
                nc.vector.tensor_scalar_add(vt_t[:], p_v[:], bqkv_sb[:, 2:3])
                p_tr = ps_gp.tile([128, RT], f32r, name="p_tr", tag="ps")
                for c4 in range(RT // 128):
                    nc.tensor.transpose(
                        p_tr[:, c4 * 128:(c4 + 1) * 128],
                        vt_t[:, c4 * 128:(c4 + 1) * 128],
                        ident[:],
                    )
                for c4 in range(RT // 128):
                    g = rt * (RT // 128) + c4
                    base = g * 130
                    for h in range(HPC):
                        nc.vector.tensor_copy(
                            v_sb[:, base + h * 65: base + h * 65 + 64],
                            p_tr[:, c4 * 128 + h * 64: c4 * 128 + h * 64 + 64],
                        )
                        nc.vector.tensor_copy(
                            v_sb[:, base + h * 65 + 64: base + h * 65 + 65],
                            ones32[:],
                        )

            # ================= phase 2: causal attention =================
            for b in range(B):
                for qb in range(N_QB):
                    qsl = slice(b * T + qb * QB, b * T + (qb + 1) * QB)
                    p_y = [
                        ps_acc.tile([65, QB], f32, name=f"p_y{h}", tag="py")
                        for h in range(HPC)
                    ]
                    n_kt = 4 * (qb + 1)
                    for kt in range(n_kt):
                        g = b * N_KT_B + kt
                        ksl = slice(g * KTILE, (g + 1) * KTILE)
                        diag = kt - 4 * qb  # >= 0 on diagonal tiles
                        st = kt == 0
                        sp = kt == n_kt - 1
                        for h in range(HPC):
                            hsl = slice(h * DK, (h + 1) * DK)
                            p_s = ps_s.tile([128, QB], f32, name="p_s", tag="ps_att")
                            nc.tensor.matmul(
                                p_s[:], kt_sb[hsl, ksl], qt_sb[hsl, qsl],
                                start=True, stop=True,
                            )
                            e_t = att.tile([128, QB], f32r, name="e_t", tag="et")
                            nc.scalar.activation(e_t[:], p_s[:], ACTF.Exp, scale=float(SCALE))
                            if diag >= 0:
                                nc.vector.tensor_tensor(
                                    out=e_t[:], in0=e_t[:],
                                    in1=msk[:, diag * QB:(diag + 1) * QB], op=MUL,
                                )
                            vbase = g * 130 + h * 65
                            nc.tensor.matmul(
                                p_y[h][:], v_sb[:, vbase:vbase + 65], e_t[:],
                                start=st, stop=sp,
                            )
                    # normalize: y / sum  (sum is row 64 of p_y)
                    yns = []
                    for h in range(HPC):
                        rr = rrp.tile([1, QB], f32, name="rr", tag="rr")
                        nc.vector.reciprocal(rr[:], p_y[h][64:65, :])
                        bc = bcp.tile([DK, QB], f32, name="bc", tag="bc")
                        nc.gpsimd.partition_broadcast(bc[:], rr[:])
                        yn_h = ynp.tile([DK, QB], f32r, name=f"yn{h}", tag=f"yn{h}")
                        nc.vector.tensor_tensor(
                            out=yn_h[:], in0=p_y[h][0:DK, :], in1=bc[:], op=MUL,
                        )
                        yns.append(yn_h)
                    # ============ phase 3: c_proj partial for these rows ============
                    for oc in range(D // 128):
                        osl = slice(oc * 128, (oc + 1) * 128)
                        p_o = ps_gp.tile([128, QB], f32, name="p_o", tag="ps")
                        nc.tensor.matmul(
                            p_o[:], wplo_sb[:, osl], yns[0][:], start=True, stop=False
                        )
                        nc.tensor.matmul(
                            p_o[:], wphi_sb[:, osl], yns[1][:], start=False, stop=True
                        )
                        ot = osp.tile([128, QB], f32, name="ot", tag="ot")
                        nc.vector.tensor_scalar_add(ot[:], p_o[:], bp_sb[:, oc:oc + 1])
                        nc.sync.dma_start(out=outT[osl, qsl], in_=ot[:])

            for _pool in (ps_acc, ps_s, ps_gp, rrp, osp, bcp, ynp, att, vts, xa):
                _pool.release()
            _free_v(); _free_kt(); _free_qt()
            cst.release()

    nc.compile()
    return nc


_CACHED = None


def _get_program():
    global _CACHED
    if _CACHED is None:
        _CACHED = build_program()
    return _CACHED


def make_in_maps(x, W_qkv, b_qkv, W_proj, b_proj):
    x = np.asarray(x, dtype=np.float32)
    W_qkv = np.asarray(W_qkv, dtype=np.float32)
    b_qkv = np.asarray(b_qkv, dtype=np.float32)
    W_proj = np.asarray(W_proj, dtype=np.float32)
    b_proj = np.asarray(b_proj, dtype=np.float32)

    xT = round_f32r(x.reshape(ROWS, D).T)
    in_maps = []
    for c in range(N_CORES):
        ch = c * CPC  # channel offset of this core's heads
        wq_c = round_f32r(W_qkv[:, ch:ch + CPC])
        wk_c = round_f32r(W_qkv[:, D + ch:D + ch + CPC])
        wv_c = round_f32r(W_qkv[:, 2 * D + ch:2 * D + ch + CPC])
        wp_c = round_f32r(W_proj[ch:ch + CPC, :])
        bqkv_c = np.stack(
            [b_qkv[ch:ch + CPC], b_qkv[D + ch:D + ch + CPC], b_qkv[2 * D + ch:2 * D + ch + CPC]],
            axis=1,
        ).astype(np.float32)
        # b_proj added once (core 0 only); partials are summed on host
        bp_c = (
            np.ascontiguousarray(b_proj.reshape(D // 128, 128).T)
            if c == 0
            else np.zeros((128, D // 128), np.float32)
        )
        in_maps.append(
            {
                "xT": xT,
                "wq": np.ascontiguousarray(wq_c),
                "wk": np.ascontiguousarray(wk_c),
                "wv": np.ascontiguousarray(wv_c),
                "wplo": np.ascontiguousarray(wp_c[0:DK, :]),
                "wphi": np.ascontiguousarray(wp_c[DK:CPC, :]),
                "bqkv": np.ascontiguousarray(bqkv_c),
                "bp": np.ascontiguousarray(bp_c.astype(np.float32)),
            }
        )
    return in_maps


def run(nc, in_maps, trace=False, trace_kwargs=None):
    from concourse.bass_utils import run_bass_kernel_spmd

    return run_bass_kernel_spmd(
        nc,
        in_maps,
        core_ids=list(range(N_CORES)),
        trace=trace,
        **(trace_kwargs or {}),
    )


def gather_output(results):
    acc = results[0]["outT"].astype(np.float32)
    for r in results[1:]:
        acc = acc + r["outT"]
    return np.ascontiguousarray(acc.T).reshape(B, T, D)


def kernel(x, W_qkv, b_qkv, W_proj, b_proj):
    nc = _get_program()
    in_maps = make_in_maps(x, W_qkv, b_qkv, W_proj, b_proj)
    res = run(nc, in_maps, trace=False)
    return gather_output(res.results)
